# revision 21
# baseline (speedup 1.0000x reference)
"""DeepseekV2 MLA attention prefill kernel for 8 Trainium2 NeuronCores.

Sharding: 2-way data-parallel over batch x 4-way tensor-parallel over heads
(4 heads per core).  The q down-projection + RMSNorm is computed on an S/4
slice per core inside each batch group and exchanged with one in-group
AllGather (1.5MB payload); the cheap compressed-KV path is replicated at
full S on every core and computed while the gather is in flight, hiding the
collective latency.  KV decompression is fused per seq chunk so compressed
KV never persists.  Per-head up-projections, attention and the output
projection are computed locally; o_proj partial sums are written bf16 and
reduced on the host during unsharding.

Layouts: activations are feature-major ([D, S]) throughout; attention
scores are computed transposed ([s_k, s_q]) so the PV matmul needs no
transposes.  RoPE is applied via host-side permuted/sign-folded weight
columns.  Matmuls run in bf16 with fp32 PSUM accumulation.  All ScalarE
transcendentals stay inside the natural_log_exp table set
(rstd = exp(-0.5 ln(ms+eps)); 1/sum = exp(-ln(sum))) so no ACT table
reloads occur mid-kernel.  Weights are loaded with few large DMAs; x is
loaded once per seq chunk; o_proj is interleaved per seq chunk with staged
writeback.
"""
import sys
sys.path.insert(0, "/opt/trn_rl_repo")

import math
import numpy as np
import ml_dtypes

import concourse.bass as bass
import concourse.tile as tile
from concourse import bacc, mybir
from concourse.bass_utils import run_bass_kernel_spmd

# ---- problem constants (hardcoded; kernel.py must be self-contained) ----
B, S, HID, H = 2, 2048, 2048, 16
Q_LORA, KV_LORA = 1536, 512
D_NOPE, D_ROPE, D_V = 128, 64, 128
D_Q = D_NOPE + D_ROPE
EPS = 1e-6
ROPE_THETA = 10000.0
N_CORES = 8
HPC = 4                      # heads per core
GROUPS = [[0, 1, 2, 3], [4, 5, 6, 7]]

PLAN_B = True
G_ROWS = Q_LORA

F32 = mybir.dt.float32
BF16 = mybir.dt.bfloat16
MM_DT = BF16

SCALE = 1.0 / math.sqrt(D_Q)

_CACHE = {}


KC = HID // 128              # 16 contraction tiles for HID
QKC = Q_LORA // 128          # 12 contraction tiles for Q_LORA
CKC = KV_LORA // 128         # 4 contraction tiles for KV_LORA


# ---------------------------------------------------------------- builder --
def build_kernel(plan_b=PLAN_B, mm_dt=MM_DT):
    nc = bacc.Bacc("TRN2", target_bir_lowering=False, debug=False,
                   num_devices=N_CORES)

    xt = nc.dram_tensor("xt", [HID, S], mm_dt, kind="ExternalInput")
    xt_loc = nc.dram_tensor("xt_loc", [HID, 512], mm_dt, kind="ExternalInput")
    wdq = nc.dram_tensor("wdq", [HID, Q_LORA], mm_dt, kind="ExternalInput")
    wuq = nc.dram_tensor("wuq", [Q_LORA, HPC * 256], mm_dt, kind="ExternalInput")
    wkva = nc.dram_tensor("wkva", [HID, KV_LORA + 2 * D_ROPE], mm_dt, kind="ExternalInput")
    wkvb = nc.dram_tensor("wkvb", [KV_LORA, HPC, 256], mm_dt, kind="ExternalInput")
    ow = nc.dram_tensor("ow", [HPC, D_V, HID], mm_dt, kind="ExternalInput")
    cos_f = nc.dram_tensor("cos_f", [D_ROPE, S], mm_dt, kind="ExternalInput")
    sin_f = nc.dram_tensor("sin_f", [D_ROPE, S], mm_dt, kind="ExternalInput")
    masks = nc.dram_tensor("masks", [4, 128, 512], mm_dt, kind="ExternalInput")
    out_t = nc.dram_tensor("out_t", [HID, S], mm_dt, kind="ExternalOutput")

    with tile.TileContext(nc) as tc:
        import contextlib
        ctx = contextlib.ExitStack()
        with ctx:
            persist = ctx.enter_context(tc.tile_pool(name="persist", bufs=1))
            wpool = ctx.enter_context(tc.tile_pool(name="wpool", bufs=5))
            spool = ctx.enter_context(tc.tile_pool(name="spool", bufs=2))
            xpool = ctx.enter_context(tc.tile_pool(name="xpool", bufs=5))
            ppool = ctx.enter_context(tc.tile_pool(name="ppool", bufs=2, space="PSUM"))
            pscore = ctx.enter_context(tc.tile_pool(name="pscore", bufs=2, space="PSUM"))
            pctx = ctx.enter_context(tc.tile_pool(name="pctx", bufs=2, space="PSUM"))
            psums = ctx.enter_context(tc.tile_pool(name="psums", bufs=2, space="PSUM"))
            dram = ctx.enter_context(tc.tile_pool(name="dram", bufs=1, space="DRAM"))

            # first-MM-gating loads: xt_loc pieces + wdq mg0 pieces (emitted
            # first so the q down-proj starts as soon as possible)
            def load_pieces(dram_t, col0, tag, pool):
                out = []
                for j in range(4):
                    t = pool.tile([128, 4, 512], mm_dt, tag=tag)
                    nc.sync.dma_start(
                        out=t,
                        in_=dram_t.ap()[j * 512:(j + 1) * 512, col0:col0 + 512]
                        .rearrange("(kc p) n -> p kc n", p=128))
                    out.append(t)
                return out

            xl_sb = load_pieces(xt_loc, 0, "xl", xpool)
            wdq_sb = {0: load_pieces(wdq, 0, "wdq", wpool)}

            # ---- constants + persistent weights (DMA queues fill in bg) ----
            wkva_sb = persist.tile([128, KC, 640], mm_dt, tag="wkva")
            nc.sync.dma_start(out=wkva_sb,
                              in_=wkva.ap().rearrange("(kc p) c -> p kc c", p=128))
            ones_sb = persist.tile([128, 1], mm_dt, tag="ones")
            nc.vector.memset(ones_sb, 1.0)
            eps_sb = persist.tile([1, 1], F32, tag="eps")
            nc.vector.memset(eps_sb, EPS)
            mask_sb = persist.tile([128, 4, 512], mm_dt, tag="masks")
            nc.sync.dma_start(out=mask_sb, in_=masks.ap().rearrange("d p c -> p d c"))
            cosf_sb = persist.tile([D_ROPE, 4, 512], mm_dt, tag="cosf")
            sinf_sb = persist.tile([D_ROPE, 4, 512], mm_dt, tag="sinf")
            nc.sync.dma_start(out=cosf_sb, in_=cos_f.ap().rearrange("d (c n) -> d c n", c=4))
            nc.sync.dma_start(out=sinf_sb, in_=sin_f.ap().rearrange("d (c n) -> d c n", c=4))

            wkvb_sb = persist.tile([128, CKC, HPC, 256], mm_dt, tag="wkvb")
            nc.sync.dma_start(out=wkvb_sb,
                              in_=wkvb.ap().rearrange("(kc p) h c -> p kc h c", p=128))
            ow_sb = persist.tile([D_V, HPC, HID], mm_dt, tag="ow")
            nc.sync.dma_start(out=ow_sb, in_=ow.ap().rearrange("h p c -> p h c"))

            # gather buffers (DRAM), split in halves so the q up-proj can
            # start on the first piece while the second is in flight
            HG = G_ROWS // 2
            g_ins = [dram.tile([HG, 512], mm_dt, name="g_in%d" % i)
                     for i in range(2)]
            g_outs = [dram.tile([4 * HG, 512], mm_dt, name="g_out%d" % i)
                      for i in range(2)]

            # ---- stage 1a: q down-proj + RMSNorm on the local S chunk ----
            qnorm_own = spool.tile([128, QKC, 512], mm_dt, tag="qnorm_own", bufs=1)
            ssq_q = psums.tile([1, 512], F32, tag="p_sum", name="ssq_q")
            for mg in range(3):
                if mg not in wdq_sb:
                    wdq_sb[mg] = load_pieces(wdq, mg * 512, "wdq", wpool)
                wts = wdq_sb[mg]
                accs = [ppool.tile([128, 512], F32, tag="p_a", name="acc_q")
                        if j < 2 else
                        pscore.tile([128, 512], F32, tag="p_sc", name="acc_q2")
                        for j in range(4)]
                for k in range(KC):
                    for j in range(4):
                        nc.tensor.matmul(
                            accs[j], wts[k // 4][:, k % 4, j * 128:(j + 1) * 128],
                            xl_sb[k // 4][:, k % 4, :],
                            start=(k == 0), stop=(k == KC - 1))
                for j in range(4):
                    m = mg * 4 + j
                    nc.vector.tensor_copy(qnorm_own[:, m, :], accs[j])
                    sq = spool.tile([128, 512], mm_dt, tag="sq", bufs=1)
                    nc.vector.tensor_tensor(sq, qnorm_own[:, m, :], qnorm_own[:, m, :],
                                            mybir.AluOpType.mult)
                    nc.tensor.matmul(ssq_q, ones_sb, sq,
                                     start=(m == 0), stop=(m == QKC - 1),
                                     skip_group_check=True)
            # rstd = exp(-0.5 * ln(ms + eps)) — stays in natural_log_exp set
            ls = spool.tile([1, 512], F32, tag="lsum", bufs=1)
            nc.scalar.activation(out=ls, in_=ssq_q,
                                 func=mybir.ActivationFunctionType.Ln,
                                 bias=eps_sb, scale=1.0 / Q_LORA)
            rstd = spool.tile([1, 512], F32, tag="rstd", bufs=1)
            nc.scalar.activation(out=rstd, in_=ls, scale=-0.5,
                                 func=mybir.ActivationFunctionType.Exp)
            rstd_bc = spool.tile([128, 512], F32, tag="bcast", bufs=1)
            nc.gpsimd.partition_broadcast(rstd_bc, rstd)
            for m in range(QKC):
                nc.vector.tensor_tensor(qnorm_own[:, m, :], qnorm_own[:, m, :],
                                        rstd_bc, mybir.AluOpType.mult)
            for t in range(2):
                nc.sync.dma_start(
                    out=g_ins[t].rearrange("(m p) n -> p m n", p=128),
                    in_=qnorm_own[:, 6 * t:6 * (t + 1), :])

            # ---- stage 1b: AllGather q_norm within batch groups ----
            for t in range(2):
                nc.gpsimd.collective_compute(
                    "AllGather", mybir.AluOpType.bypass,
                    replica_groups=GROUPS,
                    ins=[g_ins[t].opt()], outs=[g_outs[t].opt()])

            # ---- stage 1c (overlaps gather): compressed KV at full S,
            # fused with per-chunk KV decompression ----
            kpe_sb = persist.tile([D_ROPE, 4, 512], mm_dt, tag="kpe")
            kn_sb = persist.tile([D_NOPE, HPC, 4, 512], mm_dt, tag="kn")
            v_sb = persist.tile([128, S // 128, HPC * D_V], mm_dt, tag="v")

            for nch in range(4):
                xn = load_pieces(xt, nch * 512, "xl", xpool)
                ckv = spool.tile([128, CKC, 512], mm_dt, tag="ckv", bufs=2)
                ssq = psums.tile([1, 512], F32, tag="p_sum", name="ssq_kv")
                accs = [ppool.tile([128, 512], F32, tag="p_a", name="acc_kv")
                        if j < 2 else
                        pscore.tile([128, 512], F32, tag="p_sc", name="acc_kv2")
                        for j in range(5)]
                for k in range(KC):
                    for j in range(5):
                        nc.tensor.matmul(
                            accs[j], wkva_sb[:, k, j * 128:(j + 1) * 128],
                            xn[k // 4][:, k % 4, :],
                            start=(k == 0), stop=(k == KC - 1))
                for j in range(4):
                    nc.vector.tensor_copy(ckv[:, j, :], accs[j])
                    sq = spool.tile([128, 512], mm_dt, tag="sq", bufs=1)
                    nc.vector.tensor_tensor(sq, ckv[:, j, :], ckv[:, j, :],
                                            mybir.AluOpType.mult)
                    nc.tensor.matmul(ssq, ones_sb, sq,
                                     start=(j == 0), stop=(j == CKC - 1),
                                     skip_group_check=True)
                # rope chunk [E(64) | R(64)] -> kpe
                t0 = spool.tile([D_ROPE, 512], F32, tag="ropet0", bufs=1)
                t1 = spool.tile([D_ROPE, 512], F32, tag="ropet1", bufs=1)
                nc.vector.tensor_tensor(t0, accs[4][0:D_ROPE, :],
                                        cosf_sb[:, nch, :], mybir.AluOpType.mult)
                nc.vector.tensor_tensor(t1, accs[4][D_ROPE:2 * D_ROPE, :],
                                        sinf_sb[:, nch, :], mybir.AluOpType.mult)
                nc.vector.tensor_tensor(kpe_sb[:, nch, :], t0, t1,
                                        mybir.AluOpType.add)
                ls2 = spool.tile([1, 512], F32, tag="lsum", bufs=1)
                nc.scalar.activation(out=ls2, in_=ssq,
                                     func=mybir.ActivationFunctionType.Ln,
                                     bias=eps_sb, scale=1.0 / KV_LORA)
                rstd2 = spool.tile([1, 512], F32, tag="rstd", bufs=1)
                nc.scalar.activation(out=rstd2, in_=ls2, scale=-0.5,
                                     func=mybir.ActivationFunctionType.Exp)
                rstd2_bc = spool.tile([128, 512], F32, tag="bcast", bufs=1)
                nc.gpsimd.partition_broadcast(rstd2_bc, rstd2)
                for j in range(CKC):
                    nc.vector.tensor_tensor(ckv[:, j, :], ckv[:, j, :],
                                            rstd2_bc, mybir.AluOpType.mult)

                # fused stage 2a: decompress this chunk's K_nope and V
                for h in range(HPC):
                    acc = ppool.tile([128, 512], F32, tag="p_a", name="acc_kn")
                    for k in range(CKC):
                        nc.tensor.matmul(acc, wkvb_sb[:, k, h, 0:128],
                                         ckv[:, k, :],
                                         start=(k == 0), stop=(k == CKC - 1))
                    nc.scalar.copy(kn_sb[:, h, nch, :], acc)
                for st in range(4):
                    skt = nch * 4 + st
                    acc = ppool.tile([128, 512], F32, tag="p_a", name="acc_v")
                    for k in range(CKC):
                        nc.tensor.matmul(
                            acc, ckv[:, k, st * 128:(st + 1) * 128],
                            wkvb_sb[:, k, :, 128:256],
                            start=(k == 0), stop=(k == CKC - 1))
                    nc.scalar.copy(v_sb[:, skt, :], acc)

            # ---- stage 2b/2c/2d: per-seq-chunk q up-proj + attn + o_proj ----
            for sqc in range(4):
                # stream this chunk's q_norm (post-gather) in 4 quarters
                qn_src = []
                for t in range(3):
                    qf = wpool.tile([128, 4, 512], mm_dt, tag="wdq")
                    # k-tiles 4t..4t+3 mapped onto the two gather halves
                    for (ks, ke) in (((4 * t), min(4 * t + 4, 6)),
                                     ((max(4 * t, 6)), 4 * t + 4)):
                        if ks >= ke:
                            continue
                        half, r0 = (0, ks) if ke <= 6 else (1, ks - 6)
                        nc.sync.dma_start(
                            out=qf[:, ks - 4 * t:ke - 4 * t, :],
                            in_=g_outs[half][sqc * 768 + r0 * 128:
                                             sqc * 768 + (r0 + ke - ks) * 128, :]
                            .rearrange("(m p) n -> p m n", p=128))
                    qn_src.append(qf)

                qn_t = {}
                qpe_t = {}
                for g2 in range(4):   # one head (nope + rope chunk) per pass
                    wuq_s = spool.tile([128, QKC, 256], mm_dt, tag="wuq_s", bufs=2)
                    nc.sync.dma_start(
                        out=wuq_s,
                        in_=wuq.ap()[:, g2 * 256:(g2 + 1) * 256]
                        .rearrange("(kc p) c -> p kc c", p=128))
                    accs = [ppool.tile([128, 512], F32, tag="p_a", name="acc_qup")
                            for _ in range(2)]
                    for k in range(QKC):
                        for j in range(2):
                            nc.tensor.matmul(
                                accs[j],
                                wuq_s[:, k, j * 128:(j + 1) * 128],
                                qn_src[k // 4][:, k % 4, :],
                                start=(k == 0), stop=(k == QKC - 1))
                    h = g2
                    qt = spool.tile([D_NOPE, 512], mm_dt, tag="qn_h%d" % h, bufs=1)
                    nc.scalar.copy(qt, accs[0])
                    qn_t[h] = qt
                    t0 = spool.tile([D_ROPE, 512], F32, tag="ropet0", bufs=1)
                    t1 = spool.tile([D_ROPE, 512], F32, tag="ropet1", bufs=1)
                    nc.vector.tensor_tensor(t0, accs[1][0:D_ROPE, :],
                                            cosf_sb[:, sqc, :], mybir.AluOpType.mult)
                    nc.vector.tensor_tensor(t1, accs[1][D_ROPE:2 * D_ROPE, :],
                                            sinf_sb[:, sqc, :], mybir.AluOpType.mult)
                    qpt = spool.tile([D_ROPE, 512], mm_dt, tag="qpe_h%d" % h, bufs=1)
                    nc.vector.tensor_tensor(qpt, t0, t1, mybir.AluOpType.add)
                    qpe_t[h] = qpt

                n_skt = 4 * (sqc + 1)
                ctx_sb = spool.tile([D_V, HPC, 512], mm_dt, tag="ctx", bufs=1)
                recips = {}
                fin_pend = None   # (h, sum_acc, ctx_acc): finalize 1 head behind

                def finalize(fh, fsum, fctx):
                    # 1/sum via exp(-ln(sum)) on ScalarE; ctx evacuation on DVE
                    lsm = spool.tile([1, 512], F32, tag="lsum2", bufs=1)
                    nc.scalar.activation(out=lsm, in_=fsum,
                                         func=mybir.ActivationFunctionType.Ln)
                    rc = spool.tile([1, 512], F32, tag="recip", bufs=4)
                    nc.scalar.activation(out=rc, in_=lsm, scale=-1.0,
                                         func=mybir.ActivationFunctionType.Exp)
                    recips[fh] = rc
                    nc.vector.tensor_copy(ctx_sb[:, fh, :], fctx)

                for h in range(HPC):
                    sum_acc = psums.tile([1, 512], F32, tag="p_sum", name="sum_acc")
                    ctx_acc = pctx.tile([D_V, 512], F32, tag="p_ctx")
                    pending = []   # 2-deep pipeline: exp tiles awaiting sums/PV
                    for skt in range(n_skt):
                        sc = pscore.tile([128, 512], F32, tag="p_sc", name="sc")
                        nc.tensor.matmul(
                            sc, kn_sb[:, h, skt // 4, (skt % 4) * 128:(skt % 4) * 128 + 128],
                            qn_t[h], start=True, stop=False, skip_group_check=True)
                        nc.tensor.matmul(
                            sc, kpe_sb[:, skt // 4, (skt % 4) * 128:(skt % 4) * 128 + 128],
                            qpe_t[h], start=False, stop=True, skip_group_check=True)
                        ex = spool.tile([128, 512], mm_dt, tag="exp%d" % (skt % 3), bufs=1)
                        nc.scalar.activation(out=ex, in_=sc,
                                             func=mybir.ActivationFunctionType.Exp,
                                             scale=SCALE)
                        if skt >= 4 * sqc:
                            nc.vector.tensor_tensor(ex, ex, mask_sb[:, skt - 4 * sqc, :],
                                                    mybir.AluOpType.mult)
                        pending.append((ex, skt))

                        def pop_pair(last):
                            # PV for both chunks; one paired sum MM (the pair
                            # is added on DVE, halving ones-matmul count)
                            (exa, ska), (exb, skb) = pending.pop(0), pending.pop(0)
                            for pex, pskt in ((exa, ska), (exb, skb)):
                                nc.tensor.matmul(
                                    ctx_acc, v_sb[:, pskt, h * D_V:(h + 1) * D_V],
                                    pex, start=(pskt == 0), stop=last and pex is exb,
                                    skip_group_check=True)
                            pp = spool.tile([128, 512], mm_dt, tag="pp%d" % (ska % 4 // 2),
                                            bufs=1)
                            nc.vector.tensor_tensor(pp, exa, exb, mybir.AluOpType.add)
                            nc.tensor.matmul(sum_acc, ones_sb, pp,
                                             start=(ska == 0), stop=last,
                                             skip_group_check=True)

                        if len(pending) == 4:
                            pop_pair(False)
                        if skt == 1 and fin_pend is not None:
                            finalize(*fin_pend)
                            fin_pend = None
                    while pending:
                        pop_pair(len(pending) == 2)
                    fin_pend = (h, sum_acc, ctx_acc)
                finalize(*fin_pend)
                fin_pend = None

                for h in range(HPC):
                    rb = spool.tile([128, 512], F32, tag="bcast", bufs=1)
                    nc.gpsimd.partition_broadcast(rb, recips[h])
                    nc.vector.tensor_tensor(ctx_sb[:, h, :], ctx_sb[:, h, :], rb,
                                            mybir.AluOpType.mult)

                # o_proj for this seq chunk (partial sums over local heads)
                for og in range(4):
                    ostage = spool.tile([128, 4, 512], mm_dt, tag="ostage", bufs=1)
                    for hc in range(4):
                        hidc = og * 4 + hc
                        acc = ppool.tile([128, 512], F32, tag="p_a", name="acc_o")
                        for h in range(HPC):
                            nc.tensor.matmul(acc, ow_sb[:, h, hidc * 128:(hidc + 1) * 128],
                                             ctx_sb[:, h, :],
                                             start=(h == 0), stop=(h == HPC - 1))
                        nc.scalar.copy(ostage[:, hc, :], acc)
                    nc.sync.dma_start(
                        out=out_t.ap()[og * 512:(og + 1) * 512,
                                       sqc * 512:(sqc + 1) * 512]
                        .rearrange("(hc p) n -> p hc n", p=128),
                        in_=ostage)

    nc.compile()
    return nc


# ------------------------------------------------------------- host side --
def _rope_tables():
    inv_freq = 1.0 / (ROPE_THETA ** (np.arange(0, D_ROPE, 2, dtype=np.float64) / D_ROPE))
    t = np.arange(S, dtype=np.float64)
    freqs = np.outer(t, inv_freq)                    # [S, 32]
    emb = np.concatenate([freqs, freqs], axis=-1)    # [S, 64]
    return (np.cos(emb).astype(np.float32).T.copy(),
            np.sin(emb).astype(np.float32).T.copy())  # [64, S]


_E_PERM = np.concatenate([np.arange(0, D_ROPE, 2), np.arange(1, D_ROPE, 2)])


def _rope_expand(Wpe):
    """[n, 64] rope weight cols -> [n, 128]: [even/odd-reordered | rot-half signed]."""
    Y = Wpe[:, _E_PERM]
    R = np.concatenate([-Y[:, D_ROPE // 2:], Y[:, :D_ROPE // 2]], axis=1)
    return np.concatenate([Y, R], axis=1)


def _prep_inputs(hidden_states, w_dq, q_a_ln_w, w_uq, kv_a_w, kv_a_ln_w, kv_b_w, o_w,
                 plan_b=PLAN_B):
    bf = ml_dtypes.bfloat16
    cosT, sinT = _rope_tables()

    wuq_eff = (np.asarray(q_a_ln_w)[:, None] * np.asarray(w_uq)).reshape(Q_LORA, H, D_Q)
    head_blocks = []
    for h in range(H):
        head_blocks.append(np.concatenate(
            [wuq_eff[:, h, :D_NOPE], _rope_expand(wuq_eff[:, h, D_NOPE:])], axis=1))
    wuq_x = np.stack(head_blocks, axis=1)            # [1536, 16, 256]

    kv_a = np.asarray(kv_a_w)
    wkva_x = np.concatenate([kv_a[:, :KV_LORA], _rope_expand(kv_a[:, KV_LORA:])],
                            axis=1).astype(bf)       # [2048, 640]
    wkvb_eff = (np.asarray(kv_a_ln_w)[:, None] * np.asarray(kv_b_w)).reshape(KV_LORA, H, 256)
    ow_r = np.asarray(o_w).reshape(H, D_V, HID)

    c_idx = np.arange(512)[None, :]
    r_idx = np.arange(128)[:, None]
    masks = np.stack([(c_idx >= 128 * dd + r_idx) for dd in range(4)]).astype(bf)

    wdq_b = np.asarray(w_dq).astype(bf)
    hs = np.asarray(hidden_states)

    in_maps = []
    for c in range(N_CORES):
        b, hg = c // 4, c % 4
        s0 = 512 * hg
        xt_full = np.ascontiguousarray(hs[b].T).astype(bf)
        in_maps.append({
            "xt": xt_full,
            "xt_loc": np.ascontiguousarray(xt_full[:, s0:s0 + 512]),
            "wdq": wdq_b,
            "wuq": np.ascontiguousarray(
                wuq_x[:, HPC * hg: HPC * (hg + 1), :].reshape(Q_LORA, HPC * 256)).astype(bf),
            "wkva": wkva_x,
            "wkvb": np.ascontiguousarray(
                wkvb_eff[:, HPC * hg: HPC * (hg + 1)]).astype(bf),
            "ow": np.ascontiguousarray(ow_r[HPC * hg: HPC * (hg + 1)]).astype(bf),
            "cos_f": cosT.astype(bf),
            "sin_f": sinT.astype(bf),
            "masks": masks,
        })
    return in_maps


def _postprocess(results):
    out = np.empty((B, S, HID), dtype=np.float32)
    for b in range(B):
        acc = results[4 * b]["out_t"].astype(np.float32)
        for c in GROUPS[b][1:]:
            acc = acc + results[c]["out_t"].astype(np.float32)
        out[b] = acc.T
    return out


def kernel(**inputs):
    key = (PLAN_B, str(MM_DT))
    if key not in _CACHE:
        _CACHE[key] = build_kernel(PLAN_B, MM_DT)
    nc = _CACHE[key]
    in_maps = _prep_inputs(**inputs, plan_b=PLAN_B)
    r = run_bass_kernel_spmd(nc, in_maps, core_ids=list(range(N_CORES)))
    return _postprocess(r.results)
